# revision 12
# baseline (speedup 1.0000x reference)
"""3-layer GCN (gcn_norm + 3x gcn_conv + softmax) on 8 Trainium2 NeuronCores.

Strategy (self-contained; shapes hardcoded for N=16384, E=524288):
  - Node (row) sharding: core d owns nodes [d*2048, (d+1)*2048).
  - Stage 1: P1 = x @ W1 computed from a host-transposed, fp8-e3m4-cast x
    shard (the contraction dim lands on SBUF partitions; e3m4 keeps |x|<=15.5
    exactly in range and fp32 PSUM accumulation makes the only error the
    input rounding, ~3e-4 on the final softmax).
  - Both big streams (x^T and the count matrix) are pre-tiled on the host
    into partition-major layout so every DMA has 16KB contiguous runs per
    partition.
  - Aggregation out = D^-1/2 (A + I) D^-1/2 (h W): the normalization is
    folded into per-node scales dis = deg^-1/2 applied before/after a plain
    *count* aggregation.  The count matrix (A + I) is dense fp8-e4m3 per-core
    column shard [16384, 2048]; small-integer counts are exact in fp8.
  - All accumulation matmuls are PE *column-tiled*: the stationary operand
    (features or W1 k-slices) is only 64/32/16 wide, so 2 (F=64) or 4
    (F<=32) independent source tiles run concurrently in disjoint 32/64-col
    strips of the 128x128 array (tile_position), each streaming its own
    rhs.  A single full-width zeros matmul opens each PSUM bank (clears
    has_written for all 128 partitions); the strips then accumulate with
    start=False and are folded by DVE/ACT at the end.
  - The last RB source blocks of the count matrix stay resident in SBUF
    (loaded once, in space recycled from the stage-1 x^T stream via scoped
    tile pools); each layer streams only the remaining blocks.
  - A-stream DMAs ride only the sync+scalar HWDGE rings; collectives and the
    post-collective spread ride gpsimd, so an in-flight AllGather never
    head-of-line-blocks the next layer's A prefetch.
  - After each W-mul, per-core node shards of dis*(h@W) (fp16) are AllGathered
    so every core holds the full source-side operand in SBUF.
  - Layer epilogues (fold + dis scale, bias, relu) run on DVE/ACT; final
    softmax over the 16 classes runs per 128-node tile along the free dim.

kernel(**inputs) takes the FULL inputs and returns the FULL [16384, 16] fp32
output.
"""

import numpy as np

import concourse.bass as bass
import concourse.mybir as mybir
import concourse.tile as tile
from concourse import bacc
from concourse.bass_utils import run_bass_kernel_spmd
from concourse.masks import make_identity
from concourse.tile import add_dep_helper

N = 16384
NCORES = 8
CP = N // NCORES          # 2048 nodes per core
F1, F2, F3 = 64, 32, 16
KT = N // 128             # 128 k-tiles in stage 1
ST = N // 128             # 128 source tiles in aggregation
MT = CP // 128            # 16 m-tiles (local node tiles)
NCHUNK = CP // 512        # 4 free-dim chunks of 512
KB = 8                    # k-tiles per stage-1 DMA chunk
SB = 8                    # source tiles per resident A tile
ASB = 16                  # source tiles per streamed A-block DMA
RB = 4                    # resident A tiles (of SB each) across layers
NST = ST - RB * SB        # source tiles streamed per layer (104)
NAB = NST // ASB          # streamed A-blocks per layer (8)

F32 = mybir.dt.float32
F16 = mybir.dt.float16
FP8 = mybir.dt.float8e4
FP8X = mybir.dt.float8e3
NP_FP8 = mybir.dt.np(FP8)
NP_FP8X = mybir.dt.np(FP8X)

_prog_cache = {}


def _build_program():
    nc = bacc.Bacc("TRN2", target_bir_lowering=False, debug=False,
                   num_devices=NCORES)

    # xt_d flat layout: [p, kc, b, t] with feature = kc*1024 + b*128 + p
    # (b in [0,8)), node = core*2048 + t.
    xt_d = nc.dram_tensor("xt_d", [128, N * CP // 128], FP8X,
                          kind="ExternalInput")
    # a_d flat layout: [p, jb, bb, t] row = (jb*8+bb)*128 + p of the permuted
    # count matrix, t = local target.
    a_d = nc.dram_tensor("a_d", [128, N * CP // 128], FP8,
                         kind="ExternalInput")
    w1_d = nc.dram_tensor("w1_d", [128, KT * F1], F16, kind="ExternalInput")
    w2_d = nc.dram_tensor("w2_d", [F1, F2], F32, kind="ExternalInput")
    w3_d = nc.dram_tensor("w3_d", [F2, F3], F32, kind="ExternalInput")
    b1_d = nc.dram_tensor("b1_d", [F1, 1], F32, kind="ExternalInput")
    b2_d = nc.dram_tensor("b2_d", [F2, 1], F32, kind="ExternalInput")
    b3_d = nc.dram_tensor("b3_d", [F3, 1], F32, kind="ExternalInput")
    # dis16_d[p, m] = dis[core*2048 + p*16 + m] (per-partition scalars per m-tile)
    dis16_d = nc.dram_tensor("dis16_d", [128, MT], F32, kind="ExternalInput")
    # disrep_d[r, t] = dis[core*2048 + t] replicated over 64 rows
    disrep_d = nc.dram_tensor("disrep_d", [F1, CP], F16, kind="ExternalInput")
    out_d = nc.dram_tensor("out_d", [CP, F3], F32, kind="ExternalOutput")

    # DRAM bounce buffers for the AllGathers (in: Local, out: Shared)
    ps_in = {}
    ps_out = {}
    for li, F in ((1, F1), (2, F2), (3, F3)):
        ps_in[li] = nc.dram_tensor(f"ps_in_{li}", [CP, F], F16)
        ps_out[li] = nc.dram_tensor(f"ps_out_{li}", [N, F], F16,
                                    addr_space="Shared")

    with tile.TileContext(nc) as tc:
        with tc.tile_pool(name="const", bufs=1) as cpool, \
             tc.tile_pool(name="aadj", bufs=2) as apool, \
             tc.tile_pool(name="ps", bufs=1) as pspool, \
             tc.tile_pool(name="work", bufs=2) as wpool, \
             tc.tile_pool(name="psum", bufs=2, space="PSUM") as psum, \
             tc.tile_pool(name="psum_acc", bufs=1, space="PSUM") as psum_acc:

            # ---- constants -------------------------------------------------
            warm_sb = cpool.tile([128, 512], F16, tag="warm_src")
            nc.vector.memset(warm_sb[:], 0.001)
            w2_sb = cpool.tile([F1, F2], F32, tag="w2")
            nc.scalar.dma_start(out=w2_sb[:], in_=w2_d[:, :])
            w3_sb = cpool.tile([F2, F3], F32, tag="w3")
            nc.scalar.dma_start(out=w3_sb[:], in_=w3_d[:, :])
            b_sb = {}
            for li, (bd, F) in ((1, (b1_d, F1)), (2, (b2_d, F2)), (3, (b3_d, F3))):
                b_sb[li] = cpool.tile([F, 1], F32, tag=f"b{li}", name=f"b{li}_sb")
                nc.scalar.dma_start(out=b_sb[li][:], in_=bd[:, :])
            dis16_sb = cpool.tile([128, MT], F32, tag="dis16")
            nc.scalar.dma_start(out=dis16_sb[:], in_=dis16_d[:, :])
            disrep_sb = cpool.tile([F1, CP], F16, tag="disrep")
            nc.gpsimd.dma_start(out=disrep_sb[:], in_=disrep_d[:, :])
            ident = cpool.tile([128, 128], F32, tag="ident")
            make_identity(nc, ident[:])
            zw_sb = cpool.tile([128, 128], F16, tag="zw")
            nc.vector.memset(zw_sb[:], 0.0)

            def strided_m(t, m):
                """[F, CP] tile -> [F, 128] slice holding nodes q*16+m."""
                return t[:].rearrange("f (q m) -> f m q", m=MT)[:, m, :]

            warm_ctr = [0]

            def emit_warmers(after_inst, n):
                """Dummy matmuls chained after `after_inst` to keep the PE
                HAM un-throttled across a collective stall."""
                wt = psum.tile([F1, 512], F32, tag="warm",
                               name=f"warm{warm_ctr[0]}")
                warm_ctr[0] += 1
                prev = after_inst
                for i in range(n):
                    mm = nc.tensor.matmul(wt[:], lhsT=warm_sb[:, :F1],
                                          rhs=warm_sb[:],
                                          start=True, stop=True)
                    add_dep_helper(mm.ins, prev.ins, sync=(i == 0),
                                   reason="pe warmer")
                    prev = mm

            # ---- stage 1: P1T = (x @ W1)^T for local nodes ----------------
            # col-tiled pairs: even k -> psum rows 0:64, odd k -> 64:128
            rings = [nc.sync, nc.scalar, nc.gpsimd]
            arings = [nc.sync, nc.scalar]
            p1t_ps = [psum_acc.tile([128, 512], F32, tag=f"acc{c}",
                                    name=f"p1t_ps{c}")
                      for c in range(NCHUNK)]
            wt_s1 = psum.tile([F1, 512], F32, tag="warm", name="warm_s1")
            for c in range(NCHUNK):
                nc.tensor.matmul(p1t_ps[c][:], lhsT=zw_sb[:],
                                 rhs=warm_sb[:],
                                 start=True, stop=False,
                                 skip_group_check=True)
            with tc.tile_pool(name="xt", bufs=4) as xtpool:
                # w1 lives in the stage-1 scoped pool: its space is recycled
                # for the resident A-blocks afterwards.  Loaded head-first so
                # the first chunk's matmuls only wait on KB k-tiles.
                w1_sb = xtpool.tile([128, KT * F1], F16, tag="w1", bufs=1)
                nc.scalar.dma_start(out=w1_sb[:, :KB * F1],
                                    in_=w1_d[:, :KB * F1])
                for kc in range(KT // KB):
                    xt_tile = xtpool.tile([128, KB, CP], FP8X, tag="xt")
                    off = kc * KB * CP
                    if kc == 0:
                        # split the first chunk across both HWDGE rings: the
                        # first matmuls gate on it, one queue is ~170 GB/s
                        for si in range(4):
                            so = si * (KB // 4)
                            arings[si % 2].dma_start(
                                out=xt_tile[:, so:so + KB // 4, :],
                                in_=xt_d[:, off + so * CP:
                                         off + (so + KB // 4) * CP]
                                    .rearrange("p (b t) -> p b t", b=KB // 4))
                    else:
                        arings[kc % 2].dma_start(
                            out=xt_tile[:],
                            in_=xt_d[:, off:off + KB * CP]
                                .rearrange("p (b t) -> p b t", b=KB))
                    if kc == 0:
                        # progressive w1 load: each piece lands just before
                        # the chunks that need it, between xt chunks in the
                        # scalar queue
                        nc.scalar.dma_start(
                            out=w1_sb[:, KB * F1:4 * KB * F1],
                            in_=w1_d[:, KB * F1:4 * KB * F1])
                    elif kc == 1:
                        nc.scalar.dma_start(out=w1_sb[:, 4 * KB * F1:],
                                            in_=w1_d[:, 4 * KB * F1:])
                    for bp in range(KB // 2):
                        for c in range(NCHUNK):
                            for half in range(2):
                                b = bp * 2 + half
                                k = kc * KB + b
                                nc.tensor.matmul(
                                    p1t_ps[c][64 * half:64 * (half + 1), :],
                                    lhsT=w1_sb[:, k * F1:(k + 1) * F1],
                                    rhs=xt_tile[:, b, c * 512:(c + 1) * 512],
                                    start=False, stop=(k >= KT - 2),
                                    skip_group_check=True,
                                    tile_position=(0, 64 * half))
                    # filler matmuls: split the DMA-wait gap below the HAM
                    # MID window so the PE clock never re-throttles mid-stage
                    for _ in range(4):
                        nc.tensor.matmul(wt_s1[:], lhsT=warm_sb[:, :F1],
                                         rhs=warm_sb[:],
                                         start=True, stop=True)

            # resident A-blocks (last RB source blocks): loaded once into the
            # space the x^T stream just vacated, reused by every layer
            with tc.tile_pool(name="ares", bufs=1) as rpool:

                def fold_acc(acc, F, g, c, out_sb, sl, li):
                    """Fold g column-group partial sums (strips [32|64]*q of
                    `acc`) into out_sb[:, sl], applying dis scale."""
                    stride = 128 // g
                    parts = []
                    for q in range(1, g):
                        cp_sb = wpool.tile([F, 512], F32, tag=f"fc{q}",
                                           bufs=1, name=f"fold{li}_{c}_{q}")
                        nc.scalar.activation(
                            cp_sb[:], acc[stride * q:stride * q + F, :],
                            mybir.ActivationFunctionType.Identity)
                        parts.append(cp_sb)
                    zt = wpool.tile([F, 512], F32, tag="zt",
                                    bufs=1, name=f"fz{li}_{c}")
                    nc.vector.tensor_tensor(zt[:], acc[0:F, :], parts[0][:],
                                            mybir.AluOpType.add)
                    for q in range(1, g - 1):
                        nc.vector.tensor_tensor(zt[:], zt[:], parts[q][:],
                                                mybir.AluOpType.add)
                    nc.vector.tensor_tensor(out_sb[:, sl], zt[:],
                                            disrep_sb[:F, sl],
                                            mybir.AluOpType.mult)

                p1t_sb = rpool.tile([F1, CP], F32, tag="hT", bufs=1)
                for c in range(NCHUNK):
                    sl = slice(c * 512, (c + 1) * 512)
                    lo_sb = wpool.tile([F1, 512], F32, tag="fc1",
                                       bufs=1, name=f"s1fold_{c}")
                    nc.scalar.activation(lo_sb[:], p1t_ps[c][64:128, :],
                                         mybir.ActivationFunctionType.Identity)
                    nc.vector.tensor_tensor(p1t_sb[:, sl], p1t_ps[c][0:64, :],
                                            lo_sb[:], mybir.AluOpType.add)
                ps_local1 = wpool.tile([128, MT, F1], F16, tag="psl1", bufs=1)
                for m in range(MT):
                    pt = psum.tile([128, F1], F32, tag="wmul")
                    nc.tensor.transpose(pt[:], strided_m(p1t_sb, m),
                                        ident[:F1, :F1])
                    nc.vector.tensor_scalar_mul(ps_local1[:, m, :], pt[:],
                                                dis16_sb[:, m:m + 1])
                bdma1 = nc.sync.dma_start(
                    out=ps_in[1].ap().rearrange("(p m) f -> p m f", p=128),
                    in_=ps_local1[:])
                emit_warmers(bdma1, 70)
                nc.gpsimd.collective_compute(
                    "AllGather", mybir.AluOpType.bypass,
                    replica_groups=[list(range(NCORES))],
                    ins=[ps_in[1].ap().opt()],
                    outs=[ps_out[1].ap().opt()],
                )
                # resident loads ride the fast rings inside the AG1 window,
                # ahead of the layer-1 A-stream in FIFO order
                res_tiles = []
                for r in range(RB):
                    rt = rpool.tile([128, SB, CP], FP8, tag=f"ares{r}",
                                    name=f"ares{r}")
                    j0 = NST + r * SB
                    arings[r % 2].dma_start(
                        out=rt[:],
                        in_=a_d[:, j0 * CP:(j0 + SB) * CP]
                            .rearrange("p (b t) -> p b t", b=SB))
                    res_tiles.append(rt)

                def emit_wmul_scale_gather(hT_sb, F_in, F_nxt, w_sb, li):
                    """normal-land W-mul + dis scale + fp16 cast, per m-tile
                    with node-to-(partition, m) mapping n = p*16 + m; then
                    bounce to DRAM and AllGather."""
                    ps_local = wpool.tile([128, MT, F_nxt], F16,
                                          tag=f"psl{li}", bufs=1)
                    for m in range(MT):
                        pt = psum.tile([128, F_nxt], F32, tag="wmul")
                        nc.tensor.matmul(pt[:], lhsT=strided_m(hT_sb, m),
                                         rhs=w_sb[:], start=True, stop=True)
                        nc.vector.tensor_scalar_mul(ps_local[:, m, :], pt[:],
                                                    dis16_sb[:, m:m + 1])
                    bdma = nc.sync.dma_start(
                        out=ps_in[li].ap().rearrange("(p m) f -> p m f",
                                                     p=128),
                        in_=ps_local[:])
                    emit_warmers(bdma, 64)
                    nc.gpsimd.collective_compute(
                        "AllGather", mybir.AluOpType.bypass,
                        replica_groups=[list(range(NCORES))],
                        ins=[ps_in[li].ap().opt()],
                        outs=[ps_out[li].ap().opt()],
                    )
                    ps_full = pspool.tile([128, ST * F_nxt], F16,
                                          tag=f"psf{li}")
                    # dram row of (p, d, m) is d*2048 + p*16 + m; j = d*16+m
                    # chunked by core-pair so the first aggregation matmuls
                    # (low j) unblock before the whole spread lands
                    for dd in range(0, NCORES, 2):
                        nc.gpsimd.dma_start(
                            out=ps_full[:, dd * MT * F_nxt:
                                        (dd + 2) * MT * F_nxt]
                                .rearrange("p (d m f) -> p d m f", d=2, m=MT),
                            in_=ps_out[li].ap()
                                .rearrange("(d p m) f -> p d m f",
                                           d=NCORES, p=128)[:, dd:dd + 2],
                        )
                    return ps_full

                ps_full = pspool.tile([128, ST * F1], F16, tag="psf1")
                for dd in range(0, NCORES, 2):
                    nc.gpsimd.dma_start(
                        out=ps_full[:, dd * MT * F1:(dd + 2) * MT * F1]
                            .rearrange("p (d m f) -> p d m f", d=2, m=MT),
                        in_=ps_out[1].ap()
                            .rearrange("(d p m) f -> p d m f",
                                       d=NCORES, p=128)[:, dd:dd + 2],
                    )

                # ---- layers ------------------------------------------------
                for li, F in ((1, F1), (2, F2), (3, F3)):
                    g = 2 if F > 32 else 4   # column groups per PE pass
                    stride = 128 // g
                    agg_ps = [psum_acc.tile([128, 512], F32, tag=f"acc{c}",
                                            name=f"agg{li}_ps{c}")
                              for c in range(NCHUNK)]
                    for c in range(NCHUNK):
                        nc.tensor.matmul(agg_ps[c][:], lhsT=zw_sb[:],
                                         rhs=warm_sb[:],
                                         start=True, stop=False,
                                         skip_group_check=True)
                    def agg_mms(a_tile, j0, ntiles):
                        for bg in range(ntiles // g):
                            for c in range(NCHUNK):
                                for q in range(g):
                                    bb = bg * g + q
                                    j = j0 + bb
                                    nc.tensor.matmul(
                                        agg_ps[c][stride * q:stride * q + F, :],
                                        lhsT=ps_full[:, j * F:(j + 1) * F],
                                        rhs=a_tile[:, bb,
                                                   c * 512:(c + 1) * 512],
                                        start=False, stop=(j >= ST - g),
                                        skip_group_check=True,
                                        tile_position=(0, stride * q))

                    wt_a = psum.tile([F1, 512], F32, tag="warm",
                                     name=f"warm_agg{li}")
                    for ab in range(NAB):
                        a_tile = apool.tile([128, ASB, CP], FP8, tag="a")
                        arings[ab % 2].dma_start(
                            out=a_tile[:],
                            in_=a_d[:, ab * ASB * CP:(ab + 1) * ASB * CP]
                                .rearrange("p (b t) -> p b t", b=ASB))
                        agg_mms(a_tile, ab * ASB, ASB)
                        # keep the PE HAM warm across DMA-starved gaps
                        for _ in range(3):
                            nc.tensor.matmul(wt_a[:], lhsT=warm_sb[:, :F1],
                                             rhs=warm_sb[:],
                                             start=True, stop=True)
                    for r in range(RB):
                        agg_mms(res_tiles[r], NST + r * SB, SB)
                    hT_sb = rpool.tile([F, CP], F32, tag="hT", bufs=1)
                    for c in range(NCHUNK):
                        sl = slice(c * 512, (c + 1) * 512)
                        fold_acc(agg_ps[c][:], F, g, c, hT_sb, sl, li)
                    # bias + relu (identity for layer 3)
                    func = (mybir.ActivationFunctionType.Relu if li < 3
                            else mybir.ActivationFunctionType.Identity)
                    nc.scalar.activation(hT_sb[:], hT_sb[:], func,
                                         bias=b_sb[li][:, 0:1])
                    if li == 1:
                        ps_full = emit_wmul_scale_gather(hT_sb, F1, F2,
                                                         w2_sb, 2)
                    elif li == 2:
                        ps_full = emit_wmul_scale_gather(hT_sb, F2, F3,
                                                         w3_sb, 3)
                    else:
                        # transpose + batched softmax over classes (free dim)
                        h3 = wpool.tile([128, MT, F3], F32, tag="h3", bufs=1)
                        for m in range(MT):
                            pt = psum.tile([128, F3], F32, tag="wmul")
                            nc.tensor.transpose(pt[:], strided_m(hT_sb, m),
                                                ident[:F3, :F3])
                            nc.vector.tensor_copy(h3[:, m, :], pt[:])
                        mx = wpool.tile([128, MT], F32, tag="mx")
                        nc.vector.reduce_max(mx[:], h3[:],
                                             mybir.AxisListType.X,
                                             negate=True)
                        mxb = mx[:].rearrange("p (m o) -> p m o", o=1) \
                                   .broadcast_to((128, MT, F3))
                        ex = wpool.tile([128, MT, F3], F32, tag="ex", bufs=1)
                        nc.vector.tensor_tensor(ex[:], h3[:], mxb,
                                                mybir.AluOpType.add)
                        nc.scalar.activation(ex[:], ex[:],
                                             mybir.ActivationFunctionType.Exp)
                        sm = wpool.tile([128, MT], F32, tag="sm")
                        nc.vector.reduce_sum(sm[:], ex[:],
                                             mybir.AxisListType.X)
                        rc = wpool.tile([128, MT], F32, tag="rc")
                        nc.vector.reciprocal(rc[:], sm[:])
                        rcb = rc[:].rearrange("p (m o) -> p m o", o=1) \
                                   .broadcast_to((128, MT, F3))
                        o_sb = wpool.tile([128, MT, F3], F32, tag="osm",
                                          bufs=1)
                        nc.vector.tensor_tensor(o_sb[:], ex[:], rcb,
                                                mybir.AluOpType.mult)
                        nc.sync.dma_start(
                            out=out_d.ap().rearrange("(p m) c -> p m c",
                                                     p=128),
                            in_=o_sb[:])

    nc.compile()
    return nc


def _get_program():
    if "nc" not in _prog_cache:
        _prog_cache["nc"] = _build_program()
    return _prog_cache["nc"]


def _preprocess(x, edge_index, W1, b1, W2, b2, W3, b3):
    x = np.asarray(x, dtype=np.float32)
    ei = np.asarray(edge_index)
    row = ei[0].astype(np.int64)
    col = ei[1].astype(np.int64)

    deg = np.bincount(col, minlength=N).astype(np.float32) + 1.0
    dis = (1.0 / np.sqrt(deg)).astype(np.float32)

    # dense count matrix with self loops, exact small ints in fp8-e4m3
    A = np.zeros((N, N), dtype=np.uint8)
    np.add.at(A, (row, col), 1)
    idx = np.arange(N)
    A[idx, idx] += 1
    assert A.max() <= 16, "fp8 count matrix would be inexact"
    lut = np.arange(256, dtype=np.uint8).astype(np.float32) \
            .astype(NP_FP8).view(np.uint8)
    A8 = lut[A]  # uint8 bit patterns of fp8 counts

    # source-row permutation: aggregation tile j = d*16+m holds, on
    # partition p, global node d*2048 + p*16 + m
    g = np.arange(N)
    jj, pp = g // 128, g % 128
    dd, mm = jj // MT, jj % MT
    perm_src = dd * CP + pp * MT + mm
    A8p = A8[perm_src, :]

    in_maps = []
    for d in range(NCORES):
        sl = slice(d * CP, (d + 1) * CP)
        dis_d = dis[sl]
        # partition-major pre-tiling (see kernel comments for layouts)
        xtp = x[sl, :].T.astype(NP_FP8X).view(np.uint8)  # [16384 f, 2048 n]
        xtp = xtp.reshape(KT // KB, KB, 128, CP) \
                 .transpose(2, 0, 1, 3).reshape(128, N * CP // 128)
        a_sl = A8p[:, sl]                              # [16384 src, 2048 t]
        a_sl = a_sl.reshape(ST, 128, CP) \
                   .transpose(1, 0, 2).reshape(128, N * CP // 128)
        in_maps.append({
            "xt_d": np.ascontiguousarray(xtp).view(NP_FP8X),
            "a_d": np.ascontiguousarray(a_sl).view(NP_FP8),
            "w1_d": np.ascontiguousarray(
                W1.reshape(KT, 128, F1).transpose(1, 0, 2)
                  .reshape(128, KT * F1)).astype(np.float16),
            "w2_d": np.ascontiguousarray(W2, dtype=np.float32),
            "w3_d": np.ascontiguousarray(W3, dtype=np.float32),
            "b1_d": np.ascontiguousarray(b1, dtype=np.float32).reshape(F1, 1),
            "b2_d": np.ascontiguousarray(b2, dtype=np.float32).reshape(F2, 1),
            "b3_d": np.ascontiguousarray(b3, dtype=np.float32).reshape(F3, 1),
            "dis16_d": np.ascontiguousarray(dis_d.reshape(128, MT)),
            "disrep_d": np.ascontiguousarray(
                np.broadcast_to(dis_d[None, :], (F1, CP))).astype(np.float16),
        })
    return in_maps


def _execute(in_maps, trace=False, trace_cores=None):
    nc = _get_program()
    return run_bass_kernel_spmd(nc, in_maps,
                                core_ids=list(range(NCORES)), trace=trace,
                                trace_cores=trace_cores)


def kernel(x, edge_index, W1, b1, W2, b2, W3, b3):
    in_maps = _preprocess(x, edge_index, W1, b1, W2, b2, W3, b3)
    res = _execute(in_maps, trace=False)
    return np.concatenate([r["out_d"] for r in res.results], axis=0)
